# revision 1
# baseline (speedup 1.0000x reference)
"""Trainium2 Bass kernel for nn_CrossAttention (self-attention, B=1 N=4096 D=640, 8 heads x 64).

v3 sharding: 2-way sequence x 4-way head-pair. Core r = (qh, p) with qh = r // 4,
p = r % 4 owns queries [qh*2048, qh*2048+2048) and heads {2p, 2p+1} (inner dims
[128p, 128p+128)). No collectives: each core computes its pair's K/V over the
full sequence (4x less projection work than replicating all heads), runs
attention for its 2048 queries x 2 heads, and projects through its 128-row
slice of Wo. The host sums the 4 head-pair partials per sequence half (the
"all-reduce after to_out" from the sharding hint, done at unshard time) and
adds the bias.

Trick: softmax is permutation-invariant over keys, so the host sends x^T with
the core's local query half first. The SPMD program always reads queries from
columns 0:2048 — no per-core offsets, no duplicate x input.

Attention inner loop keeps the proven v2 structure: keys on partitions,
queries on the free dim, denominator fused via a ones-column in V (attn@v out
row 64), score pairs run concurrently on the two PE row-group halves, scores/
exp run LA units ahead of attn@v. New in v3:
- normalization: DVE fast-approx reciprocal on the raw denominator row
  (instead of two big ACT Ln/Exp ops after broadcast), then the DRAM-bounce
  stride-0 broadcast, then one DVE multiply per head
- output projection is staggered per query block through the next block's
  attention units, so the tail after the last attn@v is short and the PE
  never idles long enough for the HAM clock gate to re-throttle
- final PSUM->SBUF copies on DVE, keeping the ACT engine exp-only
"""

import sys
import types

sys.path.insert(0, "/opt/trn_rl_repo")

import numpy as np
import ml_dtypes


# --- reconstruct the missing antenv.axon_hooks module (NTFF profiling) ------
def _ensure_axon_hooks():
    if "antenv.axon_hooks" in sys.modules:
        return
    holder = {"hook": None}
    mod = types.ModuleType("antenv.axon_hooks")
    mod.set_axon_ntff_profile_hook = lambda h: holder.__setitem__("hook", h)
    mod.get_axon_ntff_profile_hook = lambda: holder["hook"]
    sys.modules["antenv.axon_hooks"] = mod
    try:
        import antenv

        antenv.axon_hooks = mod
    except ImportError:
        pass
    try:
        from trn_agent_boot.trn_boot import _ntff_profile_via_ctypes

        mod.set_axon_ntff_profile_hook(
            _ntff_profile_via_ctypes("/opt/axon/libaxon_pjrt.so")
        )
    except Exception:
        pass


_ensure_axon_hooks()

import concourse.bass as bass
import concourse.mybir as mybir
import concourse.tile as tile
from concourse.tile import add_dep_helper
from concourse import bass_utils
from concourse.bass import ts
from concourse.bass_utils import run_bass_kernel_spmd

# fishfood upload is unavailable in this sandbox; trace path calls it
bass_utils.upload_artifacts = lambda tmpdir: "local://" + tmpdir

BF16 = mybir.dt.bfloat16
F32 = mybir.dt.float32
bf16 = ml_dtypes.bfloat16
EXP = mybir.ActivationFunctionType.Exp

R = 8  # cores
N = 4096  # sequence length
D = 640  # model dim
H = 8  # heads
DH = 64  # head dim
QS = 2  # sequence (query) split
PS = 4  # head-pair split
NQ = N // QS  # 2048 local queries
NL = 512  # queries per query block
QB = NQ // NL  # 4 query blocks
IP = 128  # inner dims per pair (2 heads x 64)
KO = D // 128  # 5 contraction tiles for the projections
NCH = N // 128  # 32 key chunks
VW = DH + 1  # 65: v columns per head incl. the ones column
SCALE = DH**-0.5
LA = 28  # units of scores/exp lookahead ahead of attn@v


def _split_multi_waits(nc, max_waits=1):
    """walrus here rejects >1 wait per instruction; peel extras onto NoOps."""
    n = 0

    def fix(bb):
        nonlocal n
        out = []
        for ins in bb.instructions:
            blocks = getattr(ins, "blocks", None)
            if blocks:
                for b in blocks:
                    fix(b)
            si = getattr(ins, "sync_info", None)
            waits = list(si.on_wait) if (si is not None and si.on_wait) else []
            if len(waits) > max_waits:
                spill, keep = waits[:-max_waits], waits[-max_waits:]
                for w in spill:
                    out.append(
                        mybir.InstNoOp(
                            name=nc.get_next_instruction_name(),
                            engine=ins.engine,
                            sync_info=mybir.SyncInfo(on_wait=[w], on_update=[]),
                            bass_nofuse=True,
                        )
                    )
                ins.sync_info = mybir.SyncInfo(
                    on_wait=keep, on_update=list(si.on_update or [])
                )
                n += 1
            out.append(ins)
        bb.instructions = out

    for f in nc.m.functions:
        for bb in f.blocks:
            fix(bb)
    return n


def _build():
    nc = bass.Bass(num_devices=R)

    xTf = nc.dram_tensor("xTf", [D, N], BF16, kind="ExternalInput")
    wq = nc.dram_tensor("wq", [D, IP], BF16, kind="ExternalInput")
    wk = nc.dram_tensor("wk", [D, IP], BF16, kind="ExternalInput")
    wv = nc.dram_tensor("wv", [D, IP], BF16, kind="ExternalInput")
    wo = nc.dram_tensor("wo", [IP, D], BF16, kind="ExternalInput")
    out = nc.dram_tensor("out", [NQ, D], F32, kind="ExternalOutput")

    with tile.TileContext(nc) as tc:
        with (
            tc.tile_pool(name="const", bufs=1) as cp,
            tc.tile_pool(name="work", bufs=3) as wp,
            tc.tile_pool(name="atp", bufs=LA + 3) as atp,
            tc.tile_pool(name="stage", bufs=2) as sp,
            tc.tile_pool(name="ps_sc", bufs=2, space="PSUM") as ps_sc,
            tc.tile_pool(name="ps_out", bufs=2, space="PSUM") as ps_out,
            tc.tile_pool(name="ps_kp", bufs=1, space="PSUM") as ps_kp,
            tc.tile_pool(name="ps_mm", bufs=1, space="PSUM") as ps_mm,
        ):
            # ---- constants / weights / x in SBUF ---------------------------
            xt_sb = cp.tile([128, KO, N], BF16, tag="xt")
            wq_sb = cp.tile([128, KO, IP], BF16, tag="wq")
            wk_sb = cp.tile([128, KO, IP], BF16, tag="wk")
            wv_sb = cp.tile([128, KO, IP], BF16, tag="wv")
            wo_sb = cp.tile([128, D], BF16, tag="wo")

            nc.scalar.dma_start(
                wq_sb[:], wq[:].rearrange("(ko p) m -> p ko m", p=128)
            )
            nc.scalar.dma_start(
                wk_sb[:], wk[:].rearrange("(ko p) m -> p ko m", p=128)
            )
            nc.gpsimd.dma_start(
                wv_sb[:], wv[:].rearrange("(ko p) m -> p ko m", p=128)
            )
            nc.gpsimd.dma_start(wo_sb[:], wo[:])
            # one big DMA per 512-seq block (per-queue DMAs serialize on
            # completion, ~1.3us each - so few big ones, round-robined)
            xTf_r = xTf[:].rearrange("(ko p) s -> p ko s", p=128)
            xqueues = (nc.sync, nc.scalar, nc.gpsimd)
            for s in range(8):
                xqueues[s % 3].dma_start(
                    xt_sb[:, :, ts(s, NL)], xTf_r[:, :, ts(s, NL)]
                )

            warm = cp.tile([1, 520], BF16, tag="warm")
            nc.vector.memset(warm[:], 1.0)
            warm_act = cp.tile([1, 8], F32, tag="warmact")
            nc.vector.memset(warm_act[:], 1.0)
            nc.scalar.activation(warm_act[0:1, 0:1], warm_act[0:1, 1:2], EXP)
            # dummy K=1 matmuls warm the HAM clock gate while x DMAs land
            wps = ps_mm.tile([128, NL], F32, tag="mm")
            for _ in range(6):
                nc.tensor.matmul(
                    wps[0:64, :],
                    lhsT=warm[0:1, 0:64],
                    rhs=warm[0:1, 0:NL],
                    start=True,
                    stop=True,
                )

            qt_sb = cp.tile([128, QB, NL], BF16, tag="qt")  # qT [inner, q]
            ktf_sb = cp.tile([128, N], BF16, tag="ktf")  # kT [inner, keys]
            v_sb = cp.tile([128, NCH, 2 * VW], BF16, tag="v")
            projT_sb = cp.tile([128, QB, NL], BF16, tag="projT")
            odd_sb = cp.tile([64, QB, NL], BF16, tag="odd")
            ones_sb = cp.tile([65, 64], BF16, tag="ones")
            nc.vector.memset(ones_sb[:], 1.0)
            nc.vector.memset(
                v_sb[:].rearrange("p g (h w) -> p g h w", w=VW)[
                    :, :, :, DH : DH + 1
                ],
                1.0,
            )

            # ---- projection emitters ---------------------------------------
            def emit_qproj(qb):
                ps = ps_mm.tile([128, NL], F32, tag="mm")
                for ko in range(KO):
                    nc.tensor.matmul(
                        ps[:],
                        lhsT=wq_sb[:, ko, :],
                        rhs=xt_sb[:, ko, ts(qb, NL)],
                        start=(ko == 0),
                        stop=(ko == KO - 1),
                    )
                nc.vector.tensor_copy(qt_sb[:, qb, :], ps[:])

            def emit_kproj(s):
                pool, tag = (ps_kp, "kp") if s % 2 == 0 else (ps_mm, "mm")
                ps = pool.tile([128, NL], F32, tag=tag)
                for ko in range(KO):
                    nc.tensor.matmul(
                        ps[:],
                        lhsT=wk_sb[:, ko, :],
                        rhs=xt_sb[:, ko, ts(s, NL)],
                        start=(ko == 0),
                        stop=(ko == KO - 1),
                    )
                nc.vector.tensor_copy(ktf_sb[:, ts(s, NL)], ps[:])

            def emit_vproj_group(g):
                for j in range(4):
                    c = 4 * g + j
                    pool, tag = (ps_kp, "kp") if j % 2 == 0 else (ps_mm, "mm")
                    ps = pool.tile([128, NL], F32, tag=tag)
                    for ko in range(KO):
                        nc.tensor.matmul(
                            ps[:, 0:IP],
                            lhsT=xt_sb[:, ko, ts(c, 128)],
                            rhs=wv_sb[:, ko, :],
                            start=(ko == 0),
                            stop=(ko == KO - 1),
                        )
                    dst = v_sb[:, c, :].rearrange("p (h w) -> p h w", w=VW)
                    nc.vector.tensor_copy(
                        dst[:, :, 0:DH],
                        ps[:, 0:IP].rearrange("p (h d) -> p h d", d=DH),
                    )

            # ---- normalization + output projection -------------------------
            def emit_norm(qb):
                # denominator row 64 of each outp -> bf16 -> K=1 outer-product
                # matmul broadcasts it across 64 partitions (PE, not a DMA
                # bounce) -> fast-approx reciprocal (DVE) -> scale u straight
                # out of PSUM. No ACT work, no DRAM round trip, no u staging.
                outp1, outp2 = outps.pop(qb)
                den = wp.tile([65, 2, NL], F32, tag="den")
                denr = wp.tile([65, 2, NL], BF16, tag="denr")
                rb_sb = wp.tile([64, 2, NL], F32, tag="rb")
                nc.vector.tensor_copy(den[64:65, 0, :], outp1[64:65, :])
                nc.vector.tensor_copy(den[64:65, 1, :], outp2[64:65, :])
                # recip = exp(-ln(den)) on the [1, 1024] row (custom-DVE recip
                # fails codegen in this walrus; these land in ACT's
                # qb-boundary slack). ln intermediate must stay f32: a bf16
                # log-domain value turns 0.4% into ~2.5% after exp.
                nc.scalar.activation(
                    den[64:65, :, :],
                    den[64:65, :, :],
                    mybir.ActivationFunctionType.Ln,
                )
                nc.scalar.activation(
                    denr[64:65, :, :], den[64:65, :, :], EXP, scale=-1.0
                )
                rb1 = ps_kp.tile([128, NL], F32, tag="kp")
                rb2 = ps_mm.tile([128, NL], F32, tag="mm")
                for h, rbp in ((0, rb1), (1, rb2)):
                    nc.tensor.matmul(
                        rbp[0:64, :],
                        lhsT=ones_sb[64:65, :],
                        rhs=denr[64:65, h, :],
                        start=True,
                        stop=True,
                    )
                nc.vector.tensor_copy(rb_sb[:, 0, :], rb1[0:64, :])
                nc.vector.tensor_copy(rb_sb[:, 1, :], rb2[0:64, :])
                nc.vector.tensor_mul(
                    out=projT_sb[0:64, qb, :], in0=outp1[0:64, :], in1=rb_sb[:, 0, :]
                )
                nc.vector.tensor_mul(
                    out=odd_sb[:, qb, :], in0=outp2[0:64, :], in1=rb_sb[:, 1, :]
                )
                # shift odd head to partitions 64-127 (SBUF->SBUF DMA)
                nc.sync.dma_start(projT_sb[64:128, qb, :], odd_sb[:, qb, :])
                normed.add(qb)

            def emit_outproj(qb, so):
                f1 = ps_kp.tile([128, NL], F32, tag="kp")
                f2 = ps_mm.tile([128, NL], F32, tag="mm")
                nc.tensor.matmul(
                    f1[:],
                    lhsT=projT_sb[:, qb, ts(so, 128)],
                    rhs=wo_sb[:, 0:NL],
                    start=True,
                    stop=True,
                )
                nc.tensor.matmul(
                    f2[:, 0 : D - NL],
                    lhsT=projT_sb[:, qb, ts(so, 128)],
                    rhs=wo_sb[:, NL:D],
                    start=True,
                    stop=True,
                )
                o = sp.tile([128, D], F32, tag="o")
                nc.vector.tensor_copy(o[:, 0:NL], f1[:])
                nc.vector.tensor_copy(o[:, NL:D], f2[:, 0 : D - NL])
                q = nc.sync if so % 2 == 0 else nc.gpsimd
                q.dma_start(out[qb * NL + so * 128 :][0:128, :], o[:])

            # ---- attention, globally software-pipelined --------------------
            TOT = QB * NCH
            outps, at_tiles = {}, {}
            normed = set()

            def emit_attnv(j, anchor):
                qb, cj = divmod(j, NCH)
                at = at_tiles.pop(j)
                outp1, outp2 = outps[qb]
                mm1 = nc.tensor.matmul(
                    outp1[0:VW, :],
                    lhsT=v_sb[:, cj, 0:VW],
                    rhs=at[:, 0:NL],
                    start=(cj == 0),
                    stop=(cj == NCH - 1),
                )
                if anchor is not None:
                    # keep attn@v behind the lookahead scores in the PE stream
                    add_dep_helper(
                        mm1.ins, anchor, sync=False, reason="attnv after lookahead"
                    )
                nc.tensor.matmul(
                    outp2[0:VW, :],
                    lhsT=v_sb[:, cj, VW : 2 * VW],
                    rhs=at[:, NL : 2 * NL],
                    start=(cj == 0),
                    stop=(cj == NCH - 1),
                )
                if cj == NCH - 1:
                    emit_norm(qb)

            # pace projections / outproj just ahead of (resp. behind) their
            # consumers in the unit stream
            sched = {}
            for s in range(2, 8):
                sched.setdefault(4 * s - 6, []).append(("k", s))
            for g in range(8):
                sched.setdefault(16 + 4 * g, []).append(("v", g))
            for qb in range(1, QB):
                sched.setdefault(32 * qb - 12, []).append(("q", qb))
            # outproj(qb) can only be emitted after norm(qb); qb0's norm lands
            # at unit ~59 (LA-deep attnv lag), later qbs at 32(qb+1)+~10
            for so in range(4):
                sched.setdefault(60 + 3 * so, []).append(("o", 0, so))
            for qb in range(1, QB - 1):
                for so in range(4):
                    sched.setdefault(32 * (qb + 1) + 12 + 3 * so, []).append(
                        ("o", qb, so)
                    )

            emit_qproj(0)
            emit_kproj(0)
            emit_kproj(1)

            attnv_next = 0
            last_score = None
            for i in range(TOT):
                qb, c = divmod(i, NCH)
                for item in sched.get(i, ()):
                    if item[0] == "k":
                        emit_kproj(item[1])
                    elif item[0] == "v":
                        emit_vproj_group(item[1])
                    elif item[0] == "q":
                        emit_qproj(item[1])
                    elif item[1] in normed:
                        emit_outproj(item[1], item[2])
                    else:
                        # norm not emitted yet -> no dep edge would exist;
                        # retry next unit
                        sched.setdefault(i + 1, []).append(item)
                if c == 0:
                    op1 = ps_out.tile([128, NL], F32, tag="outp", name="op1")
                    op2 = ps_out.tile([128, NL], F32, tag="outp", name="op2")
                    outps[qb] = (op1, op2)
                sc = ps_sc.tile([128, 2 * NL], F32, tag="sc")
                nc.tensor.matmul(
                    sc[:, 0:NL],
                    lhsT=ktf_sb[0:64, ts(c, 128)],
                    rhs=qt_sb[0:64, qb, :],
                    start=True,
                    stop=True,
                )
                s2 = nc.tensor.matmul(
                    sc[:, NL : 2 * NL],
                    lhsT=ktf_sb[64:128, ts(c, 128)],
                    rhs=qt_sb[64:128, qb, :],
                    start=True,
                    stop=True,
                )
                last_score = s2.ins
                at = atp.tile([128, 2 * NL], BF16, tag="at")
                nc.scalar.activation(at[:], sc[:], EXP, scale=SCALE)
                at_tiles[i] = at
                lag = LA if attnv_next < NCH else (8 if attnv_next < 3 * NCH else 2)
                # cap per-unit drains: a 20-unit attnv burst at a lag
                # transition convoys the PE FIFO and starves the exp stream
                drained = 0
                while attnv_next <= i - lag and drained < 3:
                    emit_attnv(attnv_next, last_score)
                    attnv_next += 1
                    drained += 1
            while attnv_next < TOT:
                emit_attnv(attnv_next, last_score)
                attnv_next += 1

            # tail: last query block's output projection
            for so in range(4):
                emit_outproj(QB - 1, so)

    _split_multi_waits(nc)
    return nc


_NC_CACHE = {}


def _get_nc():
    if "nc" not in _NC_CACHE:
        _NC_CACHE["nc"] = _build()
    return _NC_CACHE["nc"]


def _prep_inputs(x, Wq, Wk, Wv, Wo, bo):
    x2 = np.asarray(x, dtype=np.float32).reshape(N, D)
    # qh=0 cores: natural order; qh=1 cores: local query half first (softmax
    # is permutation-invariant over keys, so K/V order just has to match)
    xT0 = np.ascontiguousarray(x2.T).astype(bf16)
    xT1 = np.ascontiguousarray(
        np.concatenate([x2[NQ:], x2[:NQ]], axis=0).T
    ).astype(bf16)
    wq_f = np.asarray(Wq, dtype=np.float32)
    wk_f = np.asarray(Wk, dtype=np.float32)
    wv_f = np.asarray(Wv, dtype=np.float32)
    wo_f = np.asarray(Wo, dtype=np.float32)
    in_maps = []
    for r in range(R):
        qh, p = divmod(r, PS)
        sl = slice(IP * p, IP * (p + 1))
        in_maps.append(
            {
                "xTf": xT0 if qh == 0 else xT1,
                "wq": np.ascontiguousarray(wq_f[:, sl]).astype(bf16),
                "wk": np.ascontiguousarray(wk_f[:, sl]).astype(bf16),
                "wv": np.ascontiguousarray(wv_f[:, sl]).astype(bf16),
                "wo": np.ascontiguousarray(wo_f[sl, :]).astype(bf16),
            }
        )
    return in_maps


def run(x, Wq, Wk, Wv, Wo, bo, trace=False):
    nc = _get_nc()
    in_maps = _prep_inputs(x, Wq, Wk, Wv, Wo, bo)
    res = run_bass_kernel_spmd(nc, in_maps, core_ids=list(range(R)), trace=trace)
    halves = []
    for qh in range(QS):
        acc = np.asarray(res.results[qh * PS]["out"], dtype=np.float32).copy()
        for p in range(1, PS):
            acc += np.asarray(res.results[qh * PS + p]["out"], dtype=np.float32)
        halves.append(acc)
    full = np.concatenate(halves, axis=0)
    full = full + np.asarray(bo, dtype=np.float32).reshape(1, D)
    return full.reshape(1, N, D), res


def kernel(x, Wq, Wk, Wv, Wo, bo):
    out, _ = run(x, Wq, Wk, Wv, Wo, bo, trace=False)
    return out

